# revision 17
# baseline (speedup 1.0000x reference)
"""Trainium2 Bass kernel for MultiHeadAttention (B=8, L=1024, D=512, H=8, Qd=64).

Sharding: data-parallel over batch B across the 8 NeuronCores (one batch
element per core).  Each core computes, for its batch element b:

    x_r  = x @ Wc.T + bc                    (pointwise conv)
    Q    = x  @ Wq.T + bq   (per head h: Q_h  [L, 64])
    K    = x_r @ Wk.T + bk
    V    = x_r @ Wv.T + bv
    S_h  = Q_h @ K_h.T / 8
    P_h  = softmax(S_h)  -> scores[b, h]    (materialized output)
    A_h  = P_h @ V_h
    out  = concat_h-interleaved(A) @ Wo.T + bo

On-chip layouts (partition dim first):
    XT, XrT, QT, KT : transposed  [D(part chunks of 128), L]
    V               : natural     [L(part chunks of 128), D]
    S.T tiles       : [128 j, 1024 l]  (feeds P.T @ V matmul via exp(S.T))
    S  tiles        : [128 l, 1024 j]  (softmax along free dim, scores output)

The PE computes out = lhsT.T @ rhs contracting over the partition dim, so
both operands of every matmul need the contraction dim on partitions; the
scores matmul is issued in BOTH orientations (K=64 each, cheap) to avoid
transposing P on chip.  Row sums for the transposed path come from a
ones-vector matmul col-packed into spare PE column groups; row sums for
the natural path come free via the activation accum_out.
"""

import numpy as np

import concourse.bass as bass
import concourse.tile as tile
from concourse import bacc, mybir
from concourse.bass_utils import run_bass_kernel_spmd
from concourse.masks import make_identity
from concourse.vector_clock import ScopedClock

F32 = mybir.dt.float32
F32R = mybir.dt.float32r
BF16 = mybir.dt.bfloat16

B, L, D = 8, 1024, 512
H, Qd = 8, 64
NCORES = 8
LC = L // 128   # 8  l-chunks
DC = D // 128   # 4  d/f-chunks
JC = L // 128   # 8  j-chunks
NH = L // 512   # 2  512-wide halves of L

USE_F32R = True
# dtype for tiles that feed fp32r matmuls: producers must write rounded f32r
MMDT = F32R if USE_F32R else F32


def _mm(ap):
    """Matmul operand cast: fp32r streams at 1 cyc/row (vs 4 for fp32)."""
    if USE_F32R and ap.dtype == F32:
        return ap.bitcast(F32R)
    return ap


class SplitDrainTileContext(tile.TileContext):
    """TileContext whose tail drain splits its sem waits across multiple
    Drain instructions -- this walrus build rejects >1 sync-wait per Drain."""

    def _drain_and_barrier(self, tick_clock, wait_clock):
        drain_inst = self.nc.sync.drain()
        wait_clock.add_sem_waits(
            drain_inst.ins, ScopedClock({None: tick_clock.global_clock})
        )
        si = drain_inst.ins.sync_info
        if si is not None and len(si.on_wait) > 1:
            extra = list(si.on_wait[1:])
            del si.on_wait[1:]
            for w in extra:
                d2 = self.nc.sync.drain()
                d2.ins.sync_info = mybir.SyncInfo(on_wait=[w], on_update=[])
        self.nc.all_engine_barrier()
        assert self.sems is not None
        popped = self.nc._tile_sem_poison_stack.pop()
        assert popped is self._sem_poison
        self.nc.clear_and_free_semaphores(list(self.sems.allocated().values()))
        self.nc.all_engine_barrier()


def _bcast_rows(ap, nrows):
    """AP reading a [n]/[1, n] DRAM row as [nrows, n] (0-stride partition).
    Only legal for DRAM sources -- SBUF partition steps must be nonzero."""
    return bass.AP(tensor=ap.tensor, offset=ap.offset,
                   ap=[[1, 1], [0, nrows]] + ap.ap[-1:])


def build_nc():
    nc = bacc.Bacc("TRN2", target_bir_lowering=False, debug=False,
                   num_devices=NCORES)

    x_in = nc.declare_dram_parameter("x", [L, D], F32, isOutput=False)
    w_ins = {
        name: nc.declare_dram_parameter(name, [D, D], F32, isOutput=False)
        for name in ("Wc", "Wq", "Wk", "Wv", "Wo")
    }
    b_ins = {
        name: nc.declare_dram_parameter(name, [D], F32, isOutput=False)
        for name in ("bc", "bq", "bk", "bv", "bo")
    }
    out_out = nc.declare_dram_parameter("out", [L, D], F32, isOutput=True)
    rs_dram = nc.dram_tensor("rs_bounce", [H, NH, 512], F32)
    scores_out = nc.declare_dram_parameter("scores", [H, L, L], F32, isOutput=True)

    from contextlib import ExitStack

    with tile.TileContext(nc) as tc:
        early = ExitStack()
        with (
            tc.tile_pool(name="persist", bufs=1) as persist,
            tc.tile_pool(name="ps_a", bufs=2, space="PSUM") as ps_a,
            tc.tile_pool(name="ps_b", bufs=2, space="PSUM") as ps_b,
            tc.tile_pool(name="ps_at", bufs=1, space="PSUM") as ps_at,
        ):
            epool = early.enter_context(tc.tile_pool(name="early", bufs=1))
            stage = early.enter_context(tc.tile_pool(name="stage", bufs=3))
            # ---------------- constants ----------------
            ident = persist.tile([128, 128], F32, name="ident", tag="ident")
            make_identity(nc, ident)
            ones_col = persist.tile([128, 1], BF16, name="ones_col", tag="ones_col")
            nc.vector.memset(ones_col, 1.0)

            # per-partition bias chunks: bias[c*128 + p] -> tile[p, c]
            bias_pp = {}
            for name in ("bc", "bq", "bk"):
                t = epool.tile([128, DC], F32, name=f"{name}_pp", tag=f"{name}_pp")
                nc.sync.dma_start(out=t, in_=b_ins[name][:].rearrange("(c p) -> p c", p=128))
                bias_pp[name] = t
            # free-dim broadcast biases
            bias_bc = {}
            for name in ("bv", "bo"):
                t = persist.tile([128, D], F32, name=f"{name}_bc", tag=f"{name}_bc")
                src = b_ins[name][:]
                nc.gpsimd.dma_start(
                    out=t, in_=bass.AP(tensor=src.tensor, offset=src.offset,
                                       ap=[[1, 1], [0, 128]] + src.ap)
                )
                bias_bc[name] = t

            # ---------------- weights: load + transpose ----------------
            # WT[w][c] = [128 d, 512 f] so that lhsT/rhs contraction dim = d.
            WT = {}
            for wname in ("Wc", "Wq", "Wk", "Wv", "Wo"):
                wpool = persist if wname == "Wo" else epool
                WT[wname] = [
                    wpool.tile([128, D], MMDT, name=f"{wname}T{c}", tag=f"{wname}T{c}")
                    for c in range(DC)
                ]
            for wname in ("Wc", "Wq", "Wk", "Wv", "Wo"):
                for r in range(DC):
                    w_nat = stage.tile([128, D], F32, name="w_nat", tag="w_nat")
                    nc.sync.dma_start(out=w_nat, in_=w_ins[wname][r * 128:(r + 1) * 128, :])
                    for c in range(DC):
                        ps = ps_a.tile([128, 128], F32, name="tps", tag="ps_a")
                        nc.tensor.transpose(ps, w_nat[:, c * 128:(c + 1) * 128], ident)
                        nc.vector.tensor_copy(WT[wname][c][:, r * 128:(r + 1) * 128], ps)

            # ---------------- x: load + transpose -> XT ----------------
            XT = [epool.tile([128, L], MMDT, name=f"XT{c}", tag=f"XT{c}")
                  for c in range(DC)]
            for lc in range(LC):
                x_nat = stage.tile([128, D], F32, name="x_nat", tag="x_nat")
                nc.sync.dma_start(out=x_nat, in_=x_in[lc * 128:(lc + 1) * 128, :])
                for c in range(DC):
                    ps = ps_a.tile([128, 128], F32, name="tps", tag="ps_a")
                    nc.tensor.transpose(ps, x_nat[:, c * 128:(c + 1) * 128], ident)
                    nc.vector.tensor_copy(XT[c][:, lc * 128:(lc + 1) * 128], ps)

            # ---------------- linear projections ----------------
            XrT = [epool.tile([128, L], MMDT, name=f"XrT{c}", tag=f"XrT{c}")
                   for c in range(DC)]
            QT = [persist.tile([128, L], MMDT, name=f"QT{c}", tag=f"QT{c}")
                  for c in range(DC)]
            KT = [persist.tile([128, L], MMDT, name=f"KT{c}", tag=f"KT{c}")
                  for c in range(DC)]

            for dst, wt, src, bias_t in (
                (XrT, WT["Wc"], XT, bias_pp["bc"]),
                (QT, WT["Wq"], XT, bias_pp["bq"]),
                (KT, WT["Wk"], XrT, bias_pp["bk"]),
            ):
                for fc in range(DC):
                    for nh in range(NH):
                        ps = ps_a.tile([128, 512], F32, name="lps", tag="ps_a")
                        for dc in range(DC):
                            nc.tensor.matmul(
                                ps,
                                _mm(wt[dc][:, fc * 128:(fc + 1) * 128]),
                                _mm(src[dc][:, nh * 512:(nh + 1) * 512]),
                                start=(dc == 0), stop=(dc == DC - 1),
                            )
                        nc.vector.tensor_scalar_add(
                            dst[fc][:, nh * 512:(nh + 1) * 512], ps,
                            bias_t[:, fc:fc + 1],
                        )

            # V natural: V[jc] = [128 j, 512 f]
            V = [persist.tile([128, D], BF16, name=f"V{jc}", tag=f"V{jc}")
                 for jc in range(JC)]
            for jc in range(JC):
                ps = ps_a.tile([128, 512], F32, name="lps", tag="ps_a")
                for dc in range(DC):
                    nc.tensor.matmul(
                        ps,
                        _mm(XrT[dc][:, jc * 128:(jc + 1) * 128]),
                        _mm(WT["Wv"][dc]),
                        start=(dc == 0), stop=(dc == DC - 1),
                    )
                nc.vector.tensor_add(V[jc], ps, bias_bc["bv"])

            # ---------------- attention (per head pair) ----------------
            early.close()  # reclaim XT/XrT/W-transpose/staging SBUF
            late = ExitStack()
            expst_pool = late.enter_context(tc.tile_pool(name="expst", bufs=2))
            pnat_pool = late.enter_context(tc.tile_pool(name="pnat", bufs=3))
            small = late.enter_context(tc.tile_pool(name="small", bufs=2))
            osb_pool = late.enter_context(tc.tile_pool(name="osb", bufs=2))
            mergedT = [persist.tile([128, L], MMDT, name=f"mgT{mc}", tag=f"mgT{mc}")
                       for mc in range(DC)]

            for pair in range(H // 2):
                hA, hB = 2 * pair, 2 * pair + 1
                ft = pair  # QT/KT chunk index; hA at partitions 0:64, hB at 64:128
                qtA, ktA = QT[ft][0:64, :], KT[ft][0:64, :]
                qtB, ktB = QT[ft][64:128, :], KT[ft][64:128, :]

                # transposed-path psums: atten accum + packed row sums
                at_ps = ps_at.tile([128, L], F32, name="at_ps", tag="at_ps")
                rs_ps = ps_a.tile([128, 512], F32, name="rs_ps", tag="ps_a")
                rs_slice = {  # (head, lh) -> (psum row, tile_position)
                    (hA, 0): (rs_ps[0:1, :], (0, 0)),
                    (hA, 1): (rs_ps[32:33, :], (0, 32)),
                    (hB, 0): (rs_ps[64:65, :], (0, 64)),
                    (hB, 1): (rs_ps[96:97, :], (0, 96)),
                }

                for jc in range(JC):
                    stA = ps_b.tile([128, L], F32, name="st_ps", tag="ps_b")
                    stB = ps_b.tile([128, L], F32, name="st_ps", tag="ps_b")
                    for lh in range(NH):
                        sl = slice(lh * 512, (lh + 1) * 512)
                        nc.tensor.matmul(
                            stA[:, sl], _mm(ktA[:, jc * 128:(jc + 1) * 128]),
                            _mm(qtA[:, sl]), start=True, stop=True)
                        nc.tensor.matmul(
                            stB[:, sl], _mm(ktB[:, jc * 128:(jc + 1) * 128]),
                            _mm(qtB[:, sl]), start=True, stop=True)
                    eA = expst_pool.tile([128, L], BF16, name="expstA", tag="expstA")
                    eB = expst_pool.tile([128, L], BF16, name="expstB", tag="expstB")
                    nc.scalar.activation(eA, stA, mybir.ActivationFunctionType.Exp,
                                         scale=0.125)
                    nc.scalar.activation(eB, stB, mybir.ActivationFunctionType.Exp,
                                         scale=0.125)
                    first, last = (jc == 0), (jc == JC - 1)
                    for lh in range(NH):
                        sl = slice(lh * 512, (lh + 1) * 512)
                        # atten: A -> at_ps[0:64], B -> at_ps[64:128]
                        nc.tensor.matmul(
                            at_ps[0:64, sl], _mm(V[jc][:, hA * 64:(hA + 1) * 64]),
                            _mm(eA[:, sl]), start=first, stop=last)
                        nc.tensor.matmul(
                            at_ps[64:128, sl], _mm(V[jc][:, hB * 64:(hB + 1) * 64]),
                            _mm(eB[:, sl]), start=first, stop=last)
                        # row sums (col-packed, M=1)
                        for head, e in ((hA, eA), (hB, eB)):
                            rs_ap, tp = rs_slice[(head, lh)]
                            nc.tensor.matmul(
                                rs_ap, _mm(ones_col), _mm(e[:, sl]),
                                start=first, stop=last, tile_position=tp)

                # natural path: softmax + scores output
                for lc in range(LC):
                    for head, qt, kt in ((hA, qtA, ktA), (hB, qtB, ktB)):
                        nat = ps_b.tile([128, L], F32, name="nat_ps", tag="ps_b")
                        for lh in range(NH):
                            sl = slice(lh * 512, (lh + 1) * 512)
                            nc.tensor.matmul(
                                nat[:, sl], _mm(qt[:, lc * 128:(lc + 1) * 128]),
                                _mm(kt[:, sl]), start=True, stop=True)
                        pn = pnat_pool.tile([128, L], F32, name="pnat", tag="pnat")
                        rs_n = small.tile([128, 1], F32, name="rs_n", tag="rs_n", bufs=6)
                        nc.scalar.activation(pn, nat,
                                             mybir.ActivationFunctionType.Exp,
                                             scale=0.125, accum_out=rs_n)
                        rc_n = small.tile([128, 1], F32, name="rc_n", tag="rc_n", bufs=6)
                        nc.vector.reciprocal(rc_n, rs_n)
                        nc.vector.tensor_scalar_mul(pn, pn, rc_n)
                        nc.sync.dma_start(
                            out=scores_out[head, lc * 128:(lc + 1) * 128, :], in_=pn)

                # finalize transposed path: scale by 1/rowsum, interleave-merge
                rs_sb = small.tile([128, 512], F32, name="rs_sb", tag="rs_sb")
                nc.vector.tensor_copy(rs_sb, rs_ps)  # rows 0/32/64/96 are live
                rsb = small.tile([128, L], F32, name="rsb", tag="rsb")
                for head, hbase in ((hA, 0), (hB, 64)):
                    for lh in range(NH):
                        p = hbase + 32 * lh
                        nc.sync.dma_start(out=rs_dram[head, lh, :],
                                          in_=rs_sb[p:p + 1, :])
                        nc.gpsimd.dma_start(
                            out=rsb[hbase:hbase + 64, lh * 512:(lh + 1) * 512],
                            in_=_bcast_rows(rs_dram[head, lh, :], 64))
                rcb = small.tile([128, L], F32, name="rcb", tag="rcb")
                nc.vector.reciprocal_approx_fast(rcb, rsb)
                asb = small.tile([128, L], MMDT, name="asb", tag="asb")
                nc.vector.tensor_mul(asb, at_ps, rcb)
                for head, base in ((hA, 0), (hB, 64)):
                    for mc in range(DC):
                        nc.sync.dma_start(
                            out=mergedT[mc][head::8, :],
                            in_=asb[base + 16 * mc:base + 16 * mc + 16, :])

            # ---------------- output projection ----------------
            for lc in range(LC):
                ps = ps_a.tile([128, 512], F32, name="lps", tag="ps_a")
                for mc in range(DC):
                    nc.tensor.matmul(
                        ps, _mm(mergedT[mc][:, lc * 128:(lc + 1) * 128]),
                        _mm(WT["Wo"][mc]), start=(mc == 0), stop=(mc == DC - 1))
                osb = osb_pool.tile([128, D], F32, name="osb", tag="osb")
                nc.vector.tensor_add(osb, ps, bias_bc["bo"])
                nc.sync.dma_start(out=out_out[lc * 128:(lc + 1) * 128, :], in_=osb)
            late.close()

    nc.compile()
    return nc


_NC_CACHE = None


def _get_nc():
    global _NC_CACHE
    if _NC_CACHE is None:
        _NC_CACHE = build_nc()
    return _NC_CACHE


def run(inputs, trace=False):
    """Run on 8 cores; returns (out, scores, BassKernelResults)."""
    nc = _get_nc()
    core_ids = list(range(NCORES))
    x = np.ascontiguousarray(np.asarray(inputs["x"], dtype=np.float32))
    shared = {}
    for name in ("Wc", "Wq", "Wk", "Wv", "Wo", "bc", "bq", "bk", "bv", "bo"):
        shared[name] = np.ascontiguousarray(np.asarray(inputs[name], dtype=np.float32))
    in_maps = [dict(shared, x=x[b]) for b in core_ids]
    res = run_bass_kernel_spmd(nc, in_maps, core_ids, trace=trace)
    out = np.stack([res.results[b]["out"] for b in core_ids])
    scores = np.stack([res.results[b]["scores"] for b in core_ids])
    return out, scores, res


def kernel(**inputs):
    out, scores, _ = run(inputs)
    return out, scores
